# revision 4
# baseline (speedup 1.0000x reference)
"""Local (sliding-window) attention kernel for Trainium2, 8 NeuronCores.

Problem: B=4, T=2048, C=1024, window=16 (17 keys per query).
    q = x@Wq.T+bq; k = x@Wk.T+bk; v = x@Wv.T+bv
    scores = (q . k_win) / sqrt(C), softmax over the +-8 window, ctx = attn . v_win
    y = ctx@Wo.T + bo

Algebraic fusion (host-side): the attention mix is linear in v and softmax rows
sum to 1, so
    scores = x M x_win.T + alpha[key] + const[row],  M = Wq.T @ Wk / sqrt(C)
    y = (P . x_win) @ W2.T + b2,  W2 = Wo @ Wv,  b2 = Wo@bv + bo
with alpha = x@(Wk.T@bq)/sqrt(C) folded into the additive mask and the
row-constant dropped (softmax-invariant).  This removes the k and v
projections entirely: the device does one input projection (zq = x@M), the
banded score product, softmax, the probability mix over raw x, and one
output projection.

Sharding: core i handles batch b = i//2, tokens [t0, t0+1024) with t0 = (i%2)*1024,
with an 8-token halo on each side (host-sliced, zero-padded at sequence edges;
validity via additive masks computed on host).

Device layout (per core, local token axis tl in [0, 1152) == global t0-8+tl):
    xT  [c, tl]    f16  (host pre-transposed, zero-padded)
    xn  [tl, c]    f16  (natural layout, 9 chunks of 128 tokens)
    zqT [co, 1024] f16  = (x@M).T for queries tl in [8, 1032)
    per 128-query block b: keys are tl in [b*128, b*128+160); scores [128, 160]
    f32 in PSUM + additive mask, exact softmax, P -> PE-transpose (128+32 rows)
    -> mix matmuls over xn -> xmixT [c, 128] -> y = xmixT.T@W2T+b2 (f16 out).
"""

import numpy as np

B, T, C = 4, 2048, 1024
P = 128
CC = C // P            # 8 channel chunks
TQ = 1024              # queries per core
TK = 1152              # padded local kv length (9 chunks)
NB = TQ // P           # 8 query blocks
WJ = 160               # key-window columns per block (128 + 32)
HALF = 8               # window // 2
SCALE = 1.0 / 32.0     # 1/sqrt(C)
N_CORES = 8
WARMUP = 40

_PROGRAM = None        # cached nc
LAST_EXEC_NS = None
TRACE = False


def _apply_tile_drain_patch():
    """walrus (CoreV3) rejects the Tile tail-drain when it carries more than a
    couple of semaphore waits ("Too many sync wait commands").  Split the waits:
    keep one on the drain, emit the rest as single-wait SP instructions."""
    import bass_rust
    import concourse.tile as tile
    from concourse.vector_clock import ScopedClock

    if getattr(tile.TileContext, "_drain_split_patch", False):
        return

    def _drain_and_barrier(self, tick_clock, wait_clock):
        nc = self.nc
        drain_inst = nc.sync.drain()
        wait_clock.add_sem_waits(
            drain_inst.ins, ScopedClock({None: tick_clock.global_clock})
        )
        si = drain_inst.ins.sync_info
        waits = list(si.on_wait)
        if len(waits) > 1:
            byid = {h.num: h for h in self.sems.allocated().values()}
            drain_inst.ins.sync_info = bass_rust.SyncInfo(
                on_wait=waits[:1], on_update=list(si.on_update)
            )
            for w in waits[1:]:
                nc.sync.wait_ge(byid[w.id], w.wait_value)

        nc.all_engine_barrier()
        assert self.sems is not None
        popped = nc._tile_sem_poison_stack.pop()
        assert popped is self._sem_poison
        nc.clear_and_free_semaphores(list(self.sems.allocated().values()))
        nc.all_engine_barrier()

    tile.TileContext._drain_and_barrier = _drain_and_barrier
    tile.TileContext._drain_split_patch = True


def _split_excess_waits(nc, limit=1):
    """This walrus build rejects instructions carrying more than a couple of
    embedded semaphore waits ("Too many sync wait commands").  Hoist excess
    waits into same-engine NoOp instructions placed immediately before."""
    import bass_rust
    import concourse.mybir as mybir

    cnt = 0
    for f in nc.m.functions:
        for bb in f.blocks:
            changed = False
            out = []
            for inst in bb.instructions:
                si = inst.sync_info
                if si is None:
                    out.append(inst)
                    continue
                waits = list(si.on_wait)
                if len(waits) > limit:
                    changed = True
                    extra, keep = waits[:-limit], waits[-limit:]
                    for i in range(0, len(extra), limit):
                        nop = mybir.InstNoOp(name=f"waitsplit_{cnt}", ins=[], outs=[])
                        cnt += 1
                        nop.engine = inst.engine
                        nop.sync_info = bass_rust.SyncInfo(
                            on_wait=extra[i: i + limit], on_update=[]
                        )
                        out.append(nop)
                    inst.sync_info = bass_rust.SyncInfo(
                        on_wait=keep, on_update=list(si.on_update)
                    )
                out.append(inst)
            if changed:
                bb.instructions = out
    return cnt


def _build_program():
    import concourse.bass as bass
    import concourse.mybir as mybir
    import concourse.tile as tile
    from concourse.masks import make_identity

    _apply_tile_drain_patch()

    dt = mybir.dt
    f16 = dt.float16
    f32 = dt.float32
    AF = mybir.ActivationFunctionType
    AX = mybir.AxisListType

    nc = bass.Bass("TRN2", target_bir_lowering=False, debug=False)

    xT_d = nc.dram_tensor("xT", [C, TK], f16, kind="ExternalInput").ap()
    xn_d = nc.dram_tensor("xn", [TK, C], f16, kind="ExternalInput").ap()
    m_d = nc.dram_tensor("m", [CC, P, CC, P], f16, kind="ExternalInput").ap()
    w2_d = nc.dram_tensor("w2", [2, P, CC, C // 2], f16, kind="ExternalInput").ap()
    b2_d = nc.dram_tensor("b2", [P, C], f32, kind="ExternalInput").ap()
    mask_d = nc.dram_tensor("mask", [NB, P, WJ], f32, kind="ExternalInput").ap()
    y_d = nc.dram_tensor("y", [TQ, C], f16, kind="ExternalOutput").ap()

    with tile.TileContext(nc) as tc:
        from contextlib import ExitStack

        with ExitStack() as ctx:
            consts = ctx.enter_context(tc.tile_pool(name="consts", bufs=1))
            qkv = ctx.enter_context(tc.tile_pool(name="qkv", bufs=1))
            work = ctx.enter_context(tc.tile_pool(name="work", bufs=3))
            ctxp = ctx.enter_context(tc.tile_pool(name="ctxp", bufs=2))
            ptp = ctx.enter_context(tc.tile_pool(name="ptp", bufs=4))
            yp = ctx.enter_context(tc.tile_pool(name="yp", bufs=2))
            ps_big = ctx.enter_context(tc.tile_pool(name="ps_big", bufs=2, space="PSUM"))
            ps_s = ctx.enter_context(tc.tile_pool(name="ps_s", bufs=2, space="PSUM"))
            ps_pt = ctx.enter_context(tc.tile_pool(name="ps_pt", bufs=2, space="PSUM"))
            ps_ct = ctx.enter_context(tc.tile_pool(name="ps_ct", bufs=2, space="PSUM"))

            # ---- persistent SBUF tensors ----
            m_sb = consts.tile([P, CC, C], f16, tag="m")
            w2_sb = consts.tile([P, CC, C], f16, tag="w2")
            xT_sb = consts.tile([P, CC, TK], f16, tag="xT")
            b2_sb = consts.tile([P, C], f32, tag="b2")
            mask_sb = consts.tile([P, NB, WJ], f32, tag="mask")
            ident = consts.tile([P, P], f16, tag="ident")

            zqT_sb = qkv.tile([P, CC, TQ], f16, tag="zqT")
            xn_sb = qkv.tile([P, TK // P, C], f16, tag="xn")

            # scratch for PE warmup; memset first so vector does it immediately
            scratch = consts.tile([P, 512], f16, tag="scratch")
            nc.vector.memset(scratch[:], 0.0)
            make_identity(nc, ident[:])

            # ---- DMAs, ordered by first compute use: m co-slices interleaved
            # with xT token-chunks (zq phase), then xn (mix), mask (softmax),
            # w2 (output projection), b2 last ----
            xT_r = xT_d.rearrange("(cc p) t -> cc p t", p=P)
            tok_chunks = [(0, 320), (320, 640), (640, 896), (896, 1152)]

            def dma_m(j):
                nc.sync.dma_start(m_sb[:, :, j * P:(j + 1) * P], m_d[j])

            def dma_xT(ci):
                lo, hi = tok_chunks[ci]
                for cc in range(CC):
                    nc.sync.dma_start(xT_sb[:, cc, lo:hi], xT_r[cc][:, lo:hi])

            dma_m(0)
            dma_xT(0)
            dma_m(1)
            dma_m(2)
            dma_xT(1)
            dma_m(3)
            dma_m(4)
            dma_xT(2)
            dma_m(5)
            dma_m(6)
            dma_xT(3)
            dma_m(7)
            xn_r = xn_d.rearrange("(ch p) c -> ch p c", p=P)
            for ch in range(TK // P):
                nc.sync.dma_start(xn_sb[:, ch, :], xn_r[ch])
            nc.sync.dma_start(mask_sb[:], mask_d.rearrange("b p j -> p b j"))
            for hb in range(2):
                nc.sync.dma_start(
                    w2_sb[:, :, hb * (C // 2):(hb + 1) * (C // 2)], w2_d[hb]
                )
            nc.sync.dma_start(b2_sb[:], b2_d[:])

            # PE warmup on the scratch tile: fills the initial DMA wait with
            # discarded matmuls so HAM un-throttles before the real work.
            ps_w = ps_big.tile([P, 512], f32, tag="big", name="ps_warm")
            for i in range(WARMUP):
                nc.tensor.matmul(
                    ps_w[:, :128],
                    lhsT=scratch[:, 0:128],
                    rhs=scratch[:, 0:128],
                    start=(i == 0),
                    stop=(i == WARMUP - 1),
                )

            # ---- zq projection: zqT[co, t] for the 1024 queries (tl offset 8),
            # four 256-token slices so early token/m chunks unblock it sooner ----
            for ts in range(4):
                for cc in range(CC):
                    ps_full = ps_big.tile([P, 512], f32, tag="big")
                    ps = ps_full[:, :256]
                    for ci in range(CC):
                        nc.tensor.matmul(
                            ps,
                            lhsT=m_sb[:, ci, cc * P:(cc + 1) * P],
                            rhs=xT_sb[:, ci, HALF + ts * 256: HALF + (ts + 1) * 256],
                            start=(ci == 0),
                            stop=(ci == CC - 1),
                        )
                    nc.scalar.activation(
                        zqT_sb[:, cc, ts * 256:(ts + 1) * 256], ps, AF.Identity
                    )

            # ---- attention + output projection, per 128-query block ----
            for b in range(NB):
                ps = ps_s.tile([P, WJ], f32, tag="s")
                for cc in range(CC):
                    nc.tensor.matmul(
                        ps,
                        lhsT=zqT_sb[:, cc, b * P:(b + 1) * P],
                        rhs=xT_sb[:, cc, b * P: b * P + WJ],
                        start=(cc == 0),
                        stop=(cc == CC - 1),
                    )
                S = work.tile([P, WJ], f32, tag="S")
                nc.vector.tensor_add(S, ps, mask_sb[:, b, :])
                negm = work.tile([P, 1], f32, tag="negm")
                nc.vector.reduce_max(negm, S, axis=AX.X, negate=True)
                P32 = work.tile([P, WJ], f32, tag="P32")
                ssum = work.tile([P, 1], f32, tag="ssum")
                nc.scalar.activation(
                    P32, S, AF.Exp, bias=negm[:, 0:1], accum_out=ssum[:, 0:1]
                )
                rr = work.tile([P, 1], f32, tag="rr")
                nc.vector.reciprocal(rr, ssum)
                P16 = work.tile([P, WJ], f16, tag="P16")
                nc.vector.tensor_scalar_mul(P16, P32, rr[:, 0:1])

                # transpose P: [128q, 160k] -> [128k,128q] + [32k,128q]
                pps0 = ps_pt.tile([P, P], f16, tag="pt")
                nc.tensor.transpose(pps0, P16[:, 0:P], ident[:])
                pt0 = ptp.tile([P, P], f16, tag="ptt")
                nc.vector.tensor_copy(pt0, pps0)
                pps1 = ps_pt.tile([P, P], f16, tag="pt")
                nc.tensor.transpose(pps1[0:32, :], P16[:, P:WJ], ident[:])
                pt1 = ptp.tile([P, P], f16, tag="ptt")
                nc.vector.tensor_copy(pt1[0:32, :], pps1[0:32, :])

                # xmixT[c, q] = sum_keys x[key, c] * P_T[key, q]
                xmixT = ctxp.tile([P, CC, P], f16, tag="xmixT")
                for cs in range(CC):
                    pc = ps_ct.tile([P, P], f32, tag="ct")
                    nc.tensor.matmul(
                        pc,
                        lhsT=xn_sb[:, b, cs * P:(cs + 1) * P],
                        rhs=pt0[:],
                        start=True,
                        stop=False,
                    )
                    nc.tensor.matmul(
                        pc,
                        lhsT=xn_sb[0:32, b + 1, cs * P:(cs + 1) * P],
                        rhs=pt1[0:32, :],
                        start=False,
                        stop=True,
                    )
                    nc.scalar.activation(xmixT[:, cs, :], pc, AF.Identity)

                y_sb = yp.tile([P, C], f16, tag="y")
                for h in range(2):
                    psy = ps_big.tile([P, 512], f32, tag="big")
                    for ci in range(CC):
                        nc.tensor.matmul(
                            psy,
                            lhsT=xmixT[:, ci, :],
                            rhs=w2_sb[:, ci, h * 512:(h + 1) * 512],
                            start=(ci == 0),
                            stop=(ci == CC - 1),
                        )
                    nc.vector.tensor_add(
                        y_sb[:, h * 512:(h + 1) * 512], psy,
                        b2_sb[:, h * 512:(h + 1) * 512],
                    )
                    nc.sync.dma_start(
                        y_d[b * P:(b + 1) * P, h * 512:(h + 1) * 512],
                        y_sb[:, h * 512:(h + 1) * 512],
                    )

    _split_excess_waits(nc)
    return nc


def _host_inputs(x, Wq, bq, Wk, bk, Wv, bv, Wo, bo):
    """Build per-core input maps (fused weight products shared across cores)."""
    f16 = np.float16
    f32 = np.float32
    Wq = np.asarray(Wq, f32)
    Wk = np.asarray(Wk, f32)
    Wv = np.asarray(Wv, f32)
    Wo = np.asarray(Wo, f32)
    bq = np.asarray(bq, f32)
    bv = np.asarray(bv, f32)
    bo = np.asarray(bo, f32)

    Mfull = (Wq.T @ Wk) * np.float32(SCALE)          # [ci, co]
    W2T = (Wo @ Wv).T                                 # [ci, co]
    b2 = Wo @ bv + bo                                 # [C]
    u = (Wk.T @ bq) * np.float32(SCALE)               # [C], alpha = x@u

    # m dram layout [co_chunk, p, ci_chunk, 128co]
    m_h = np.ascontiguousarray(
        Mfull.reshape(CC, P, CC, P).transpose(2, 1, 0, 3)
    ).astype(f16)
    # w2 dram layout [co_half, p, ci_chunk, 512co]
    w2_h = np.ascontiguousarray(
        W2T.reshape(CC, P, 2, C // 2).transpose(2, 1, 0, 3)
    ).astype(f16)
    b2_h = np.ascontiguousarray(np.broadcast_to(b2, (P, C))).astype(f32)

    x = np.asarray(x, f32)
    in_maps = []
    for core in range(N_CORES):
        bidx = core // 2
        t0 = (core % 2) * TQ
        lo = t0 - HALF
        s0 = max(lo, 0)
        s1 = min(lo + TK, T)
        xT = np.zeros((C, TK), f16)
        xT[:, s0 - lo: s1 - lo] = x[bidx, s0:s1, :].T.astype(f16)
        xn = np.zeros((TK, C), f16)
        xn[s0 - lo: s1 - lo, :] = x[bidx, s0:s1, :].astype(f16)

        alpha = x[bidx] @ u                           # [T]
        ii = np.arange(P)[None, :, None]
        jj = np.arange(WJ)[None, None, :]
        bb = np.arange(NB)[:, None, None]
        band = (jj - ii >= 0) & (jj - ii <= 2 * HALF)
        gk = lo + bb * P + jj
        valid = band & (gk >= 0) & (gk < T)
        mask = np.where(
            valid, alpha[np.clip(gk, 0, T - 1)].astype(f32), np.float32(-1e30)
        )
        mask = np.ascontiguousarray(mask, f32)

        in_maps.append(
            {
                "xT": xT,
                "xn": xn,
                "m": m_h,
                "w2": w2_h,
                "b2": b2_h,
                "mask": mask,
            }
        )
    return in_maps


def kernel(x, Wq, bq, Wk, bk, Wv, bv, Wo, bo, window):
    global _PROGRAM, LAST_EXEC_NS
    assert int(window) == 2 * HALF

    from concourse import bass_utils

    if _PROGRAM is None:
        _PROGRAM = _build_program()
    nc = _PROGRAM

    in_maps = _host_inputs(x, Wq, bq, Wk, bk, Wv, bv, Wo, bo)
    res = bass_utils.run_bass_kernel_spmd(
        nc, in_maps, core_ids=list(range(N_CORES)), trace=TRACE
    )
    LAST_EXEC_NS = res.exec_time_ns

    out = np.empty((B, T, C), np.float32)
    for core in range(N_CORES):
        bidx = core // 2
        t0 = (core % 2) * TQ
        out[bidx, t0: t0 + TQ, :] = res.results[core]["y"].astype(np.float32)
    return out
